# revision 1
# baseline (speedup 1.0000x reference)
"""Trainium2 Bass kernel for nn_CausalFMMAttention.

Reference computation (per batch n, head h — all (n,h) pairs independent):
  phi1(x) = elu(x)+1 ; phi2(x) = (elu(x)+1)^2
  Two causal linear-attention branches (feature maps phi1 / phi2, K row-normalized,
  Q normalization cancels, key_lengths cancels under K-normalization, eps negligible):
      LVb[l] = (sum_{s<=l} (Qb_l . Kbn_s) V_s) / (Qb_l . cumsum(Kbn)_l)
  plus a width-10 banded causal softmax branch:
      SV[l]  = softmax_band(Q_l . K_s / sqrt(E)) @ V
  out = W1*SV + W2*LV1 + W3*LV2

Sharding: 16 (n,h) units, 2 per core across 8 cores (data-parallel N x
tensor-parallel H). Each core runs an identical program on its own 2 units.

Implementation: chunked scan over L in chunks of 128.
  - per chunk, PE computes A^T[s,l] = K.Q for the three branches via row-tiled
    (tile_position) matmuls on transposed operands; transposed operands are
    produced on-chip with col-tiled PE transpose-matmuls.
  - causal/band masking is fused into the (mandatory) PSUM->SBUF evacuations.
  - intra-chunk A@[V|1] and inter-chunk Q@[S|Kcum] accumulate into one PSUM
    tile per 4-chunk group; a [E, D+1] running state S accumulates in PSUM
    across chunks (K^T @ [V|1] matmuls).
  - the band crosses chunk boundaries by <=9 keys: handled with a tiny extra
    matmul against the previous chunk's K-tail / V-tail.
"""

import os
import sys
from contextlib import ExitStack

import numpy as np

if "/opt/trn_rl_repo" not in sys.path:
    sys.path.insert(0, "/opt/trn_rl_repo")

import concourse.bacc as bacc
import concourse.bass as bass
import concourse.mybir as mybir
import concourse.tile as tile
from concourse.bass_utils import run_bass_kernel_spmd
from concourse.masks import make_identity

F32 = mybir.dt.float32
BF = mybir.dt.bfloat16
ALU = mybir.AluOpType
AF = mybir.ActivationFunctionType

N, L, H, E = 2, 2048, 8, 32
D = E
NCORES = 8
UPC = (N * H) // NCORES  # units per core = 2
C = 128                  # chunk length
NCH = L // C             # 16 chunks
BW = 10                  # band width
TB = BW - 1              # boundary tail size = 9
TEMP = 1.0 / np.sqrt(np.float32(E))


def _units_of_core(c):
    return [((c * UPC + i) // H, (c * UPC + i) % H) for i in range(UPC)]


# ---------------------------------------------------------------------------
# kernel body (one core: UPC units)
# ---------------------------------------------------------------------------

class _Unit:
    """Per-unit SBUF tensors + scan state."""

    def __init__(self, tc, pools, consts, q_ap, k_ap, v_ap, w_ap, o_ap, tag):
        nc = tc.nc
        ident, maskA, maskB, ones_row = consts
        (fpool, spool, qkt_pool, a_pool, araw_pool, p_pool, s_psum_pool,
         sb2_pool) = pools
        self.pools = pools
        self.consts = consts
        self.o_ap = o_ap
        self.tag = tag

        # qpack/kpack chunk layout (96 cols per chunk): [phi1 | phi2 | raw]
        self.qpack = fpool.tile([128, NCH * 96], BF, tag=f"qpack{tag}")
        self.kpack = fpool.tile([128, NCH * 96], BF, tag=f"kpack{tag}")
        self.vpu = fpool.tile([128, NCH * (D + 1)], BF, tag=f"vpu{tag}")
        self.vpsm = fpool.tile([128, NCH * (D + 1)], BF, tag=f"vpsm{tag}")
        self.qkt_all = fpool.tile([96, NCH * 256], BF, tag=f"qkt{tag}")
        self.out_sb = fpool.tile([128, NCH * E], F32, tag=f"out{tag}")
        self.wb = fpool.tile([128, 96], BF, tag=f"wb{tag}")
        self.wb32 = fpool.tile([128, 96], F32, tag=f"wb32{tag}")
        self.wrow = fpool.tile([1, 96], F32, tag=f"wrow{tag}")
        self.sc1 = spool.tile([128, NCH * E], BF, tag=f"sc1_{tag}")
        self.ssum = spool.tile([128, 2 * NCH], F32, tag=f"ssum{tag}")
        self.srec = spool.tile([128, 2 * NCH], F32, tag=f"srec{tag}")
        self.s_sb_prev = None
        self.p_ps = None

        qv = c3(self.qpack, 96)
        kv = c3(self.kpack, 96)
        self.q1r, self.q2r, self.qrr = (qv[:, :, 0:32], qv[:, :, 32:64],
                                        qv[:, :, 64:96])
        self.k1r, self.k2r, self.krr = (kv[:, :, 0:32], kv[:, :, 32:64],
                                        kv[:, :, 64:96])

        # ---------------- loads (HWDGE f32; converts happen on-engine) ----
        qd = q_ap.rearrange("(c p) e -> p c e", p=128)
        kd = k_ap.rearrange("(c p) e -> p c e", p=128)
        vd = v_ap.rearrange("(c p) e -> p c e", p=128)
        (fpool, spool) = self.pools[0:2]
        self.qf = spool.tile([128, NCH * E], F32, tag=f"qf{tag}")
        self.kf = spool.tile([128, NCH * E], F32, tag=f"kf{tag}")
        self.vf = spool.tile([128, NCH * E], F32, tag=f"vf{tag}")
        nc.sync.dma_start(out=c3(self.qf), in_=qd)
        nc.sync.dma_start(out=c3(self.kf), in_=kd)
        nc.sync.dma_start(out=c3(self.vf), in_=vd)
        nc.sync.dma_start(out=self.wrow[0:1, :],
                          in_=w_ap.rearrange("a e -> (a e)")[None, :])

    def prelude(self, tc):
        """Feature maps + W broadcast + V variants (whole unit)."""
        nc = tc.nc
        ident, maskA, maskB, ones_row = self.consts
        (fpool, spool, qkt_pool, a_pool, araw_pool, p_pool, s_psum_pool,
         sb2_pool) = self.pools
        sc1 = self.sc1

        wb_ps = qkt_pool.tile([128, 96], F32, tag="qkt_ps")
        nc.tensor.matmul(wb_ps[:, :], lhsT=ones_row[0:1, 0:128],
                         rhs=self.wrow[0:1, :], start=True, stop=True)
        nc.scalar.copy(self.wb[:, :], wb_ps[:, :])
        nc.scalar.copy(self.wb32[:, :], wb_ps[:, :])

        # raw bf16 copies for the transposes + V
        nc.vector.tensor_copy(self.qrr, c3(self.qf))
        nc.vector.tensor_copy(self.krr, c3(self.kf))
        nc.scalar.copy(c3(self.vpu, D + 1)[:, :, 0:D], c3(self.vf))
        nc.gpsimd.memset(c3(self.vpu, D + 1)[:, :, D : D + 1], 1.0)
        # phi1(x) = exp(min(x,0)) + relu(x); phi2 = phi1^2
        nc.scalar.activation(c3(sc1), c3(self.qf), AF.Exp)
        nc.vector.tensor_scalar_min(sc1[:, :], sc1[:, :], 1.0)
        nc.scalar.activation(self.q1r, c3(self.qf), AF.Relu)
        nc.vector.tensor_add(self.q1r, self.q1r, c3(sc1))
        nc.scalar.square(self.q2r, self.q1r)
        nc.scalar.activation(c3(sc1), c3(self.kf), AF.Exp)
        nc.vector.tensor_scalar_min(sc1[:, :], sc1[:, :], 1.0)
        nc.scalar.activation(self.k1r, c3(self.kf), AF.Relu)
        nc.vector.tensor_add(self.k1r, self.k1r, c3(sc1))
        nc.scalar.square(self.k2r, self.k1r)
        # K row-normalization (over E)
        nc.vector.tensor_reduce(self.ssum[:, 0:NCH], self.k1r,
                                axis=mybir.AxisListType.X, op=ALU.add)
        nc.vector.tensor_reduce(self.ssum[:, NCH : 2 * NCH], self.k2r,
                                axis=mybir.AxisListType.X, op=ALU.add)
        nc.vector.reciprocal(self.srec[:, :], self.ssum[:, :])
        r1b = (self.srec[:, None, 0:NCH].rearrange("p a c -> p c a")
               .broadcast_to([128, NCH, E]))
        r2b = (self.srec[:, None, NCH : 2 * NCH].rearrange("p a c -> p c a")
               .broadcast_to([128, NCH, E]))
        nc.vector.tensor_mul(self.k1r, self.k1r, r1b)
        nc.gpsimd.tensor_mul(self.k2r, self.k2r, r2b)

        # vpsm = V * W1 (softmax branch carries its W fold; ones col = denom)
        w1b = self.wb[:, None, 0:E].broadcast_to([128, NCH, E])
        nc.vector.tensor_mul(c3(self.vpsm, D + 1)[:, :, 0:D],
                             c3(self.vpu, D + 1)[:, :, 0:D], w1b)
        nc.gpsimd.memset(c3(self.vpsm, D + 1)[:, :, D : D + 1], 1.0)

    def pair(self, tc, c0):
        """Process chunks c0, c0+1 with paired evacuations."""
        nc = tc.nc
        ident, maskA, maskB, ones_row = self.consts
        (fpool, spool, qkt_pool, a_pool, araw_pool, p_pool, s_psum_pool,
         sb2_pool) = self.pools

        # --- transposes for both chunks into one PSUM bank ---
        qkt_ps = qkt_pool.tile([96, 512], BF, tag="qkt_ps")
        for i in (0, 1):
            p0 = 96 * (c0 + i)
            nc.tensor.transpose(qkt_ps[:, 256 * i : 256 * i + 128],
                                self.qpack[:, p0 : p0 + 96], ident[:, :])
            nc.tensor.transpose(qkt_ps[:, 256 * i + 128 : 256 * i + 256],
                                self.kpack[:, p0 : p0 + 96], ident[:, :])
        nc.scalar.copy(self.qkt_all[:, 256 * c0 : 256 * (c0 + 2)],
                       qkt_ps[:, :])

        def qt(c):
            return self.qkt_all[:, 256 * c : 256 * c + 128]

        def kt(c):
            return self.qkt_all[:, 256 * c + 128 : 256 * (c + 1)]

        # --- A matmuls (both chunks) ---
        # bank assignment is fixed per PE row group: concurrent matmuls in
        # different row groups must never share a PSUM bank (HW fault).
        a12_ps = a_pool.tile([128, 1024], F32, tag="a12_ps")
        araw_ps = araw_pool.tile([128, 512], F32, tag="araw_ps")
        for i in (0, 1):
            c = c0 + i
            nc.tensor.matmul(a12_ps[:, 128 * i : 128 * (i + 1)],
                             lhsT=kt(c)[0:32, :], rhs=qt(c)[0:32, :],
                             start=True, stop=True)
            nc.tensor.matmul(a12_ps[:, 512 + 128 * i : 512 + 128 * (i + 1)],
                             lhsT=kt(c)[32:64, :], rhs=qt(c)[32:64, :],
                             start=True, stop=True)
            nc.tensor.matmul(araw_ps[:, 256 * i : 256 * i + 128],
                             lhsT=kt(c)[64:96, :], rhs=qt(c)[64:96, :],
                             start=True, stop=True)
            if c > 0:
                # band boundary: prev-chunk keys x first TB queries (band
                # mask keeps only the tail); same row group as Araw.
                nc.tensor.matmul(araw_ps[:, 256 * i + 128 : 256 * i + 128 + TB],
                                 lhsT=kt(c - 1)[64:96, :],
                                 rhs=qt(c)[64:96, 0:TB],
                                 start=True, stop=True)
            else:
                nc.vector.memset(araw_ps[:, 128 : 128 + TB], 0.0)

        # --- paired masked evacuations ---
        a12m = sb2_pool.tile([128, 512], BF, tag="a12m")  # (b, i, 128)
        nc.vector.tensor_mul(
            a12m[:].rearrange("p (b i x) -> p b i x", b=2, x=128),
            a12_ps[:].rearrange("p (b y) -> p b y", b=2)
                [:, :, 0:256].rearrange("p b (i x) -> p b i x", x=128),
            maskA[:, None, 0:128][:, None].broadcast_to([128, 2, 2, 128]))
        eband = sb2_pool.tile([128, 2 * (128 + TB)], BF, tag="eband")
        nc.scalar.activation(
            eband[:].rearrange("p (i x) -> p i x", i=2),
            araw_ps[:].rearrange("p (i y) -> p i y", i=2)[:, :, 0 : 128 + TB],
            AF.Exp, scale=float(TEMP))
        nc.gpsimd.tensor_mul(
            eband[:].rearrange("p (i x) -> p i x", i=2),
            eband[:].rearrange("p (i x) -> p i x", i=2),
            maskB[:, None, :].broadcast_to([128, 2, 128 + TB]))

        # --- per-chunk P matmuls + state updates + group epilogue ---
        for i in (0, 1):
            c = c0 + i
            j = c % 4
            s_sb = self.s_sb_prev
            if j == 0:
                self.p_ps = p_pool.tile([128, 4 * 3 * (D + 1)], F32,
                                        tag="p_ps")
            p_ps = self.p_ps
            pc0 = 3 * (D + 1) * j

            ebm = eband[:, (128 + TB) * i : (128 + TB) * (i + 1)]
            pcol = pc0 + (D + 1) * 2
            nc.tensor.matmul(p_ps[:, pcol : pcol + D + 1],
                             lhsT=ebm[:, 0:128],
                             rhs=self.vpsm[:, (D + 1) * c : (D + 1) * (c + 1)],
                             start=(j == 0), stop=False)
            if c > 0:
                nc.tensor.matmul(
                    p_ps[0:TB, pcol : pcol + D + 1],
                    lhsT=ebm[:, 128 : 128 + TB],
                    rhs=self.vpsm[:, (D + 1) * (c - 1) : (D + 1) * c],
                    start=False, stop=False)
            for bi in range(2):
                pcol = pc0 + (D + 1) * bi
                nc.tensor.matmul(
                    p_ps[:, pcol : pcol + D + 1],
                    lhsT=a12m[:, 256 * bi + 128 * i : 256 * bi + 128 * (i + 1)],
                    rhs=self.vpu[:, (D + 1) * c : (D + 1) * (c + 1)],
                    start=False, stop=False)
                if s_sb is not None:
                    b0 = 32 * bi
                    nc.tensor.matmul(p_ps[:, pcol : pcol + D + 1],
                                     lhsT=qt(c)[b0 : b0 + 32, :],
                                     rhs=s_sb[b0 : b0 + 32, :],
                                     start=False, stop=(j == 3 and bi == 1))

            # state update: [S1; S2] += [K1n | K2n]^T @ [V | 1]
            if c < NCH - 1:
                p0 = 96 * c
                supd_ps = s_psum_pool.tile([64, D + 1], F32, tag="supd_ps")
                nc.tensor.matmul(supd_ps[:, :],
                                 lhsT=self.kpack[:, p0 : p0 + 64],
                                 rhs=self.vpu[:, (D + 1) * c : (D + 1) * (c + 1)],
                                 start=True, stop=True)
                s_sb_new = sb2_pool.tile([64, D + 1], BF, tag="s_sb")
                if c == 0:
                    nc.vector.tensor_copy(s_sb_new[:, :], supd_ps[:, :])
                else:
                    nc.vector.tensor_add(s_sb_new[:, :], self.s_sb_prev[:, :],
                                         supd_ps[:, :])
                self.s_sb_prev = s_sb_new

            # per-group epilogue: z = 1/den, out = sum_b W_b*num_b*z_b
            if j == 3:
                g = c // 4
                p4 = p_ps[:].rearrange("p (j b x) -> p j b x", j=4, x=D + 1)
                z12 = sb2_pool.tile([128, 12], F32, tag="z12")
                z4 = z12[:].rearrange("p (j b) -> p j b", j=4)
                nc.vector.reciprocal(z4[:, :, :, None],
                                     p4[:, :, :, D : D + 1])
                obig = sb2_pool.tile([128, 4 * 3 * D], F32, tag="obig")
                o4 = obig[:].rearrange("p (j b x) -> p j b x", j=4, x=D)
                nc.vector.tensor_mul(
                    o4, p4[:, :, :, 0:D],
                    z4[:, :, :, None].broadcast_to([128, 4, 3, D]))
                w23 = (self.wb32[:, None, None, E : 3 * E]
                       .rearrange("p a b (w x) -> p a (b w) x", x=D)
                       .broadcast_to([128, 4, 2, D]))
                nc.gpsimd.tensor_mul(o4[:, :, 0:2, :], o4[:, :, 0:2, :], w23)
                t1 = sb2_pool.tile([128, 4 * D], F32, tag="t1")
                t13 = t1[:].rearrange("p (j x) -> p j x", x=D)
                nc.gpsimd.tensor_add(t13, o4[:, :, 0, :], o4[:, :, 1, :])
                nc.gpsimd.tensor_add(
                    c3(self.out_sb)[:, 4 * g : 4 * (g + 1), :], t13,
                    o4[:, :, 2, :])

    def store(self, tc):
        nc = tc.nc
        od = self.o_ap.rearrange("(c p) e -> p c e", p=128)
        nc.sync.dma_start(out=od, in_=c3(self.out_sb))


def c3(t, x=E):  # [128, NCH*x] -> [128, NCH, x]
    return t[:].rearrange("p (c x) -> p c x", x=x)


def build_core_kernel(ctx, tc, outs, ins):
    """outs/ins: dicts of DRAM APs. ins: q, k, v [UPC, L, E]; w [UPC, 3, E]."""
    nc = tc.nc
    const_pool = ctx.enter_context(tc.tile_pool(name="const", bufs=1))
    fpool = ctx.enter_context(tc.tile_pool(name="fpers", bufs=1))
    spool = ctx.enter_context(tc.tile_pool(name="fscratch", bufs=1))
    qkt_pool = ctx.enter_context(tc.tile_pool(name="qkt", bufs=2, space="PSUM"))
    a_pool = ctx.enter_context(tc.tile_pool(name="aps", bufs=1, space="PSUM"))
    araw_pool = ctx.enter_context(tc.tile_pool(name="araw", bufs=1, space="PSUM"))
    p_pool = ctx.enter_context(tc.tile_pool(name="pps", bufs=2, space="PSUM"))
    s_psum_pool = ctx.enter_context(tc.tile_pool(name="spsum", bufs=1, space="PSUM"))
    sb2_pool = ctx.enter_context(tc.tile_pool(name="sb2", bufs=6))

    ident = const_pool.tile([128, 128], BF, tag="ident")
    make_identity(nc, ident[:, :])
    ones_row = const_pool.tile([1, 128], F32, tag="ones_row")
    nc.gpsimd.memset(ones_row[:, :], 1.0)

    # causal keep-mask (s <= l), duplicated along cols for both branches
    maskA = const_pool.tile([128, 256], F32, tag="maskA")
    nc.gpsimd.memset(maskA[:, :], 1.0)
    nc.gpsimd.affine_select(
        out=maskA[:, :], in_=maskA[:, :], compare_op=ALU.is_ge, fill=0.0,
        base=0, pattern=[[0, 2], [1, 128]], channel_multiplier=-1)

    # band mask: cols 0..127: 1 where 0 <= l-s <= BW-1 ; cols 128..136:
    # boundary block: keep prev-chunk key p for query l iff p >= (C-TB)+l
    maskB = const_pool.tile([128, 128 + TB], BF, tag="maskB")
    nc.gpsimd.memset(maskB[:, :], 0.0)
    nc.gpsimd.memset(maskB[:, 0:128], 1.0)
    nc.gpsimd.affine_select(
        out=maskB[:, 0:128], in_=maskB[:, 0:128], compare_op=ALU.is_ge,
        fill=0.0, base=0, pattern=[[1, 128]], channel_multiplier=-1)
    nc.gpsimd.affine_select(
        out=maskB[:, 0:128], in_=maskB[:, 0:128], compare_op=ALU.is_ge,
        fill=0.0, base=BW - 1, pattern=[[-1, 128]], channel_multiplier=1)
    nc.gpsimd.memset(maskB[:, 128 : 128 + TB], 1.0)
    nc.gpsimd.affine_select(
        out=maskB[:, 128 : 128 + TB], in_=maskB[:, 128 : 128 + TB],
        compare_op=ALU.is_ge, fill=0.0, base=-(C - TB), pattern=[[-1, TB]],
        channel_multiplier=1)

    consts = (ident, maskA, maskB, ones_row)
    pools = (fpool, spool, qkt_pool, a_pool, araw_pool, p_pool, s_psum_pool,
             sb2_pool)
    units = []
    for u in range(UPC):
        units.append(_Unit(tc, pools, consts,
                           ins["q"][u], ins["k"][u], ins["v"][u], ins["w"][u],
                           outs["o"][u], tag=u))
    for unit in units:
        unit.prelude(tc)
    # interleave the two units' chunk scans so independent work fills the
    # pipeline bubbles of each unit's serial chain
    for c0 in range(0, NCH, 2):
        for unit in units:
            unit.pair(tc, c0)
    for unit in units:
        unit.store(tc)


# ---------------------------------------------------------------------------
# host-side entry point
# ---------------------------------------------------------------------------

_CACHE = {}


def _get_nc():
    if "nc" in _CACHE:
        return _CACHE["nc"]
    nc = bacc.Bacc("TRN2", target_bir_lowering=False, debug=False,
                   enable_asserts=True, num_devices=NCORES)
    ins = {
        "q": nc.dram_tensor("q", [UPC, L, E], F32, kind="ExternalInput").ap(),
        "k": nc.dram_tensor("k", [UPC, L, E], F32, kind="ExternalInput").ap(),
        "v": nc.dram_tensor("v", [UPC, L, E], F32, kind="ExternalInput").ap(),
        "w": nc.dram_tensor("w", [UPC, 3, E], F32, kind="ExternalInput").ap(),
    }
    outs = {"o": nc.dram_tensor("o", [UPC, L, E], F32, kind="ExternalOutput").ap()}
    with tile.TileContext(nc) as tc:
        with ExitStack() as ctx:
            build_core_kernel(ctx, tc, outs, ins)
    nc.compile()
    _CACHE["nc"] = nc
    return nc


def make_in_maps(queries, keys, values, W1, W2, W3):
    in_maps = []
    for core in range(NCORES):
        units = _units_of_core(core)
        in_maps.append({
            "q": np.stack([queries[n, :, h, :] for (n, h) in units]).copy(),
            "k": np.stack([keys[n, :, h, :] for (n, h) in units]).copy(),
            "v": np.stack([values[n, :, h, :] for (n, h) in units]).copy(),
            "w": np.stack([
                np.stack([W1[0, 0, h], W2[0, 0, h], W3[0, 0, h]])
                for (n, h) in units]).copy(),
        })
    return in_maps


def kernel(**inputs):
    queries = np.asarray(inputs["queries"], dtype=np.float32)
    keys = np.asarray(inputs["keys"], dtype=np.float32)
    values = np.asarray(inputs["values"], dtype=np.float32)
    W1 = np.asarray(inputs["W1"], dtype=np.float32)
    W2 = np.asarray(inputs["W2"], dtype=np.float32)
    W3 = np.asarray(inputs["W3"], dtype=np.float32)

    nc = _get_nc()
    in_maps = make_in_maps(queries, keys, values, W1, W2, W3)
    res = run_bass_kernel_spmd(nc, in_maps, core_ids=list(range(NCORES)),
                               trace=bool(int(os.environ.get("KERNEL_TRACE", "0"))))
    _CACHE["last_results"] = res
    out = np.zeros((N, L, H, E), dtype=np.float32)
    for core in range(NCORES):
        r = res.results[core]["o"]
        for i, (n, h) in enumerate(_units_of_core(core)):
            out[n, :, h, :] = r[i]
    return out



# revision 2
# speedup vs baseline: 1.5723x; 1.5723x over previous
"""Trainium2 Bass kernel for nn_CausalFMMAttention.

Reference computation (per batch n, head h — all (n,h) pairs independent):
  phi1(x) = elu(x)+1 ; phi2(x) = (elu(x)+1)^2
  Two causal linear-attention branches (feature maps phi1 / phi2, K row-normalized,
  Q normalization cancels, key_lengths cancels under K-normalization, eps negligible):
      LVb[l] = (sum_{s<=l} (Qb_l . Kbn_s) V_s) / (Qb_l . cumsum(Kbn)_l)
  plus a width-10 banded causal softmax branch:
      SV[l]  = softmax_band(Q_l . K_s / sqrt(E)) @ V
  out = W1*SV + W2*LV1 + W3*LV2

Sharding: 16 (n,h) units, 2 per core across 8 cores (data-parallel N x
tensor-parallel H). Each core runs an identical program on its own 2 units.

Implementation: chunked scan over L in chunks of 128.
  - per chunk, PE computes A^T[s,l] = K.Q for the three branches via row-tiled
    (tile_position) matmuls on transposed operands; transposed operands are
    produced on-chip with col-tiled PE transpose-matmuls.
  - causal/band masking is fused into the (mandatory) PSUM->SBUF evacuations.
  - intra-chunk A@[V|1] and inter-chunk Q@[S|Kcum] accumulate into one PSUM
    tile per 4-chunk group; a [E, D+1] running state S accumulates in PSUM
    across chunks (K^T @ [V|1] matmuls).
  - the band crosses chunk boundaries by <=9 keys: handled with a tiny extra
    matmul against the previous chunk's K-tail / V-tail.
"""

import os
import sys
from contextlib import ExitStack

import numpy as np

if "/opt/trn_rl_repo" not in sys.path:
    sys.path.insert(0, "/opt/trn_rl_repo")

import concourse.bacc as bacc
import concourse.bass as bass
import concourse.mybir as mybir
import concourse.tile as tile
from concourse.bass_utils import run_bass_kernel_spmd
from concourse.masks import make_identity

F32 = mybir.dt.float32
BF = mybir.dt.bfloat16
ALU = mybir.AluOpType
AF = mybir.ActivationFunctionType

N, L, H, E = 2, 2048, 8, 32
D = E
NCORES = 8
UPC = (N * H) // NCORES  # units per core = 2
C = 128                  # chunk length
NCH = L // C             # 16 chunks
BW = 10                  # band width
TB = BW - 1              # boundary tail size = 9
TEMP = 1.0 / np.sqrt(np.float32(E))


def _units_of_core(c):
    return [((c * UPC + i) // H, (c * UPC + i) % H) for i in range(UPC)]


# ---------------------------------------------------------------------------
# kernel body (one core: UPC units)
# ---------------------------------------------------------------------------

class _Unit:
    """Per-unit SBUF tensors + scan state."""

    def __init__(self, tc, pools, consts, q_ap, k_ap, v_ap, w_ap, o_ap, tag):
        nc = tc.nc
        ident, maskA, maskB, ones_row = consts
        (fpool, spool, qkt_pool, a_pool, araw_pool, p_pool, s_psum_pool,
         sb2_pool) = pools
        self.pools = pools
        self.consts = consts
        self.o_ap = o_ap
        self.tag = tag

        # qpack/kpack chunk layout (96 cols per chunk): [phi1 | phi2 | raw]
        self.qpack = fpool.tile([128, NCH * 96], BF, tag=f"qpack{tag}")
        self.kpack = fpool.tile([128, NCH * 96], BF, tag=f"kpack{tag}")
        self.vpu = fpool.tile([128, NCH * (D + 1)], BF, tag=f"vpu{tag}")
        self.vpsm = fpool.tile([128, NCH * (D + 1)], BF, tag=f"vpsm{tag}")
        self.qkt_all = fpool.tile([96, NCH * 256], BF, tag=f"qkt{tag}")
        self.out_sb = fpool.tile([128, NCH * E], F32, tag=f"out{tag}")
        self.wb = fpool.tile([128, 96], BF, tag=f"wb{tag}")
        self.wb32 = fpool.tile([128, 96], F32, tag=f"wb32{tag}")
        self.wrow = fpool.tile([1, 96], F32, tag=f"wrow{tag}")
        self.sc1 = spool.tile([128, NCH * E], BF, tag=f"sc1_{tag}")
        self.ssum = spool.tile([128, 2 * NCH], F32, tag=f"ssum{tag}")
        self.srec = spool.tile([128, 2 * NCH], F32, tag=f"srec{tag}")
        self.s_sb_prev = None
        self.p_ps = None

        qv = c3(self.qpack, 96)
        kv = c3(self.kpack, 96)
        self.q1r, self.q2r, self.qrr = (qv[:, :, 0:32], qv[:, :, 32:64],
                                        qv[:, :, 64:96])
        self.k1r, self.k2r, self.krr = (kv[:, :, 0:32], kv[:, :, 32:64],
                                        kv[:, :, 64:96])

        # ---------------- loads (HWDGE f32; converts happen on-engine) ----
        qd = q_ap.rearrange("(c p) e -> p c e", p=128)
        kd = k_ap.rearrange("(c p) e -> p c e", p=128)
        vd = v_ap.rearrange("(c p) e -> p c e", p=128)
        (fpool, spool) = self.pools[0:2]
        self.qf = spool.tile([128, NCH * E], F32, tag=f"qf{tag}")
        self.kf = spool.tile([128, NCH * E], F32, tag=f"kf{tag}")
        self.vf = spool.tile([128, NCH * E], F32, tag=f"vf{tag}")
        nc.sync.dma_start(out=c3(self.qf), in_=qd)
        nc.sync.dma_start(out=c3(self.kf), in_=kd)
        nc.sync.dma_start(out=c3(self.vf), in_=vd)
        nc.sync.dma_start(out=self.wrow[0:1, :],
                          in_=w_ap.rearrange("a e -> (a e)")[None, :])

    def prelude(self, tc):
        """Feature maps + W broadcast + V variants (whole unit)."""
        nc = tc.nc
        ident, maskA, maskB, ones_row = self.consts
        (fpool, spool, qkt_pool, a_pool, araw_pool, p_pool, s_psum_pool,
         sb2_pool) = self.pools
        sc1 = self.sc1

        wb_ps = qkt_pool.tile([128, 96], F32, tag="qkt_ps")
        nc.tensor.matmul(wb_ps[:, :], lhsT=ones_row[0:1, 0:128],
                         rhs=self.wrow[0:1, :], start=True, stop=True)
        nc.scalar.copy(self.wb[:, :], wb_ps[:, :])
        nc.scalar.copy(self.wb32[:, :], wb_ps[:, :])

        # raw bf16 copies for the transposes + V
        nc.vector.tensor_copy(self.qrr, c3(self.qf))
        nc.vector.tensor_copy(self.krr, c3(self.kf))
        nc.scalar.copy(c3(self.vpu, D + 1)[:, :, 0:D], c3(self.vf))
        nc.gpsimd.memset(c3(self.vpu, D + 1)[:, :, D : D + 1], 1.0)
        # phi1(x) = exp(min(x,0)) + relu(x); phi2 = phi1^2
        nc.scalar.activation(c3(sc1), c3(self.qf), AF.Exp)
        nc.vector.tensor_scalar_min(sc1[:, :], sc1[:, :], 1.0)
        nc.scalar.activation(self.q1r, c3(self.qf), AF.Relu)
        nc.vector.tensor_add(self.q1r, self.q1r, c3(sc1))
        nc.scalar.square(self.q2r, self.q1r)
        nc.scalar.activation(c3(sc1), c3(self.kf), AF.Exp)
        nc.vector.tensor_scalar_min(sc1[:, :], sc1[:, :], 1.0)
        nc.scalar.activation(self.k1r, c3(self.kf), AF.Relu)
        nc.vector.tensor_add(self.k1r, self.k1r, c3(sc1))
        nc.scalar.square(self.k2r, self.k1r)
        # K row-normalization (over E)
        nc.vector.tensor_reduce(self.ssum[:, 0:NCH], self.k1r,
                                axis=mybir.AxisListType.X, op=ALU.add)
        nc.vector.tensor_reduce(self.ssum[:, NCH : 2 * NCH], self.k2r,
                                axis=mybir.AxisListType.X, op=ALU.add)
        nc.vector.reciprocal(self.srec[:, :], self.ssum[:, :])
        r1b = (self.srec[:, None, 0:NCH].rearrange("p a c -> p c a")
               .broadcast_to([128, NCH, E]))
        r2b = (self.srec[:, None, NCH : 2 * NCH].rearrange("p a c -> p c a")
               .broadcast_to([128, NCH, E]))
        nc.vector.tensor_mul(self.k1r, self.k1r, r1b)
        nc.gpsimd.tensor_mul(self.k2r, self.k2r, r2b)

        # vpsm = V * W1 (softmax branch carries its W fold; ones col = denom)
        w1b = self.wb[:, None, 0:E].broadcast_to([128, NCH, E])
        nc.vector.tensor_mul(c3(self.vpsm, D + 1)[:, :, 0:D],
                             c3(self.vpu, D + 1)[:, :, 0:D], w1b)
        nc.gpsimd.memset(c3(self.vpsm, D + 1)[:, :, D : D + 1], 1.0)

    def pair(self, tc, c0):
        """Process chunks c0, c0+1 with paired evacuations."""
        nc = tc.nc
        ident, maskA, maskB, ones_row = self.consts
        (fpool, spool, qkt_pool, a_pool, araw_pool, p_pool, s_psum_pool,
         sb2_pool) = self.pools

        # --- transposes for both chunks into one PSUM bank ---
        qkt_ps = qkt_pool.tile([96, 512], BF, tag="qkt_ps")
        for i in (0, 1):
            p0 = 96 * (c0 + i)
            nc.tensor.transpose(qkt_ps[:, 256 * i : 256 * i + 128],
                                self.qpack[:, p0 : p0 + 96], ident[:, :])
            nc.tensor.transpose(qkt_ps[:, 256 * i + 128 : 256 * i + 256],
                                self.kpack[:, p0 : p0 + 96], ident[:, :])
        nc.scalar.copy(self.qkt_all[:, 256 * c0 : 256 * (c0 + 2)],
                       qkt_ps[:, :])

        def qt(c):
            return self.qkt_all[:, 256 * c : 256 * c + 128]

        def kt(c):
            return self.qkt_all[:, 256 * c + 128 : 256 * (c + 1)]

        # --- A matmuls (both chunks) ---
        # bank assignment is fixed per PE row group: concurrent matmuls in
        # different row groups must never share a PSUM bank (HW fault).
        a12_ps = a_pool.tile([128, 1024], F32, tag="a12_ps")
        araw_ps = araw_pool.tile([128, 512], F32, tag="araw_ps")
        for i in (0, 1):
            c = c0 + i
            nc.tensor.matmul(a12_ps[:, 128 * i : 128 * (i + 1)],
                             lhsT=kt(c)[0:32, :], rhs=qt(c)[0:32, :],
                             start=True, stop=True)
            nc.tensor.matmul(a12_ps[:, 512 + 128 * i : 512 + 128 * (i + 1)],
                             lhsT=kt(c)[32:64, :], rhs=qt(c)[32:64, :],
                             start=True, stop=True)
            nc.tensor.matmul(araw_ps[:, 256 * i : 256 * i + 128],
                             lhsT=kt(c)[64:96, :], rhs=qt(c)[64:96, :],
                             start=True, stop=True)
            if c > 0:
                # band boundary: prev-chunk keys x first TB queries (band
                # mask keeps only the tail); same row group as Araw.
                nc.tensor.matmul(araw_ps[:, 256 * i + 128 : 256 * i + 128 + TB],
                                 lhsT=kt(c - 1)[64:96, :],
                                 rhs=qt(c)[64:96, 0:TB],
                                 start=True, stop=True)
            else:
                nc.vector.memset(araw_ps[:, 128 : 128 + TB], 0.0)

        # --- paired masked evacuations ---
        a12m = sb2_pool.tile([128, 512], BF, tag="a12m")  # (b, i, 128)
        nc.vector.tensor_mul(
            a12m[:].rearrange("p (b i x) -> p b i x", b=2, x=128),
            a12_ps[:].rearrange("p (b y) -> p b y", b=2)
                [:, :, 0:256].rearrange("p b (i x) -> p b i x", x=128),
            maskA[:, None, 0:128][:, None].broadcast_to([128, 2, 2, 128]))
        eband = sb2_pool.tile([128, 2 * (128 + TB)], BF, tag="eband")
        nc.scalar.activation(
            eband[:].rearrange("p (i x) -> p i x", i=2),
            araw_ps[:].rearrange("p (i y) -> p i y", i=2)[:, :, 0 : 128 + TB],
            AF.Exp, scale=float(TEMP))
        nc.gpsimd.tensor_mul(
            eband[:].rearrange("p (i x) -> p i x", i=2),
            eband[:].rearrange("p (i x) -> p i x", i=2),
            maskB[:, None, :].broadcast_to([128, 2, 128 + TB]))

        # --- per-chunk P matmuls + state updates + group epilogue ---
        for i in (0, 1):
            c = c0 + i
            j = c % 4
            s_sb = self.s_sb_prev
            if j == 0:
                self.p_ps = p_pool.tile([128, 4 * 3 * (D + 1)], F32,
                                        tag="p_ps")
            p_ps = self.p_ps
            pc0 = 3 * (D + 1) * j

            ebm = eband[:, (128 + TB) * i : (128 + TB) * (i + 1)]
            pcol = pc0 + (D + 1) * 2
            nc.tensor.matmul(p_ps[:, pcol : pcol + D + 1],
                             lhsT=ebm[:, 0:128],
                             rhs=self.vpsm[:, (D + 1) * c : (D + 1) * (c + 1)],
                             start=(j == 0), stop=False)
            if c > 0:
                nc.tensor.matmul(
                    p_ps[0:TB, pcol : pcol + D + 1],
                    lhsT=ebm[:, 128 : 128 + TB],
                    rhs=self.vpsm[:, (D + 1) * (c - 1) : (D + 1) * c],
                    start=False, stop=False)
            for bi in range(2):
                pcol = pc0 + (D + 1) * bi
                nc.tensor.matmul(
                    p_ps[:, pcol : pcol + D + 1],
                    lhsT=a12m[:, 256 * bi + 128 * i : 256 * bi + 128 * (i + 1)],
                    rhs=self.vpu[:, (D + 1) * c : (D + 1) * (c + 1)],
                    start=False, stop=False)
                if s_sb is not None:
                    b0 = 32 * bi
                    nc.tensor.matmul(p_ps[:, pcol : pcol + D + 1],
                                     lhsT=qt(c)[b0 : b0 + 32, :],
                                     rhs=s_sb[b0 : b0 + 32, :],
                                     start=False, stop=(j == 3 and bi == 1))

            # state update: [S1; S2] += [K1n | K2n]^T @ [V | 1]
            if c < NCH - 1:
                p0 = 96 * c
                supd_ps = s_psum_pool.tile([64, D + 1], F32, tag="supd_ps")
                nc.tensor.matmul(supd_ps[:, :],
                                 lhsT=self.kpack[:, p0 : p0 + 64],
                                 rhs=self.vpu[:, (D + 1) * c : (D + 1) * (c + 1)],
                                 start=True, stop=True)
                s_sb_new = sb2_pool.tile([64, D + 1], BF, tag="s_sb")
                if c == 0:
                    nc.vector.tensor_copy(s_sb_new[:, :], supd_ps[:, :])
                else:
                    nc.vector.tensor_add(s_sb_new[:, :], self.s_sb_prev[:, :],
                                         supd_ps[:, :])
                self.s_sb_prev = s_sb_new

            # per-group epilogue: z = 1/den, out = sum_b W_b*num_b*z_b
            if j == 3:
                g = c // 4
                p4 = p_ps[:].rearrange("p (j b x) -> p j b x", j=4, x=D + 1)
                z12 = sb2_pool.tile([128, 12], F32, tag="z12")
                z4 = z12[:].rearrange("p (j b) -> p j b", j=4)
                nc.vector.reciprocal(z4[:, :, :, None],
                                     p4[:, :, :, D : D + 1])
                obig = sb2_pool.tile([128, 4 * 3 * D], F32, tag="obig")
                o4 = obig[:].rearrange("p (j b x) -> p j b x", j=4, x=D)
                nc.vector.tensor_mul(
                    o4, p4[:, :, :, 0:D],
                    z4[:, :, :, None].broadcast_to([128, 4, 3, D]))
                w23 = (self.wb32[:, None, None, E : 3 * E]
                       .rearrange("p a b (w x) -> p a (b w) x", x=D)
                       .broadcast_to([128, 4, 2, D]))
                nc.gpsimd.tensor_mul(o4[:, :, 0:2, :], o4[:, :, 0:2, :], w23)
                t1 = sb2_pool.tile([128, 4 * D], F32, tag="t1")
                t13 = t1[:].rearrange("p (j x) -> p j x", x=D)
                nc.gpsimd.tensor_add(t13, o4[:, :, 0, :], o4[:, :, 1, :])
                nc.gpsimd.tensor_add(
                    c3(self.out_sb)[:, 4 * g : 4 * (g + 1), :], t13,
                    o4[:, :, 2, :])

    def store(self, tc):
        nc = tc.nc
        od = self.o_ap.rearrange("(c p) e -> p c e", p=128)
        nc.sync.dma_start(out=od, in_=c3(self.out_sb))


def c3(t, x=E):  # [128, NCH*x] -> [128, NCH, x]
    return t[:].rearrange("p (c x) -> p c x", x=x)


def build_core_kernel(ctx, tc, outs, ins):
    """outs/ins: dicts of DRAM APs. ins: q, k, v [UPC, L, E]; w [UPC, 3, E]."""
    nc = tc.nc
    const_pool = ctx.enter_context(tc.tile_pool(name="const", bufs=1))
    fpool = ctx.enter_context(tc.tile_pool(name="fpers", bufs=1))
    spool = ctx.enter_context(tc.tile_pool(name="fscratch", bufs=1))
    qkt_pool = ctx.enter_context(tc.tile_pool(name="qkt", bufs=2, space="PSUM"))
    a_pool = ctx.enter_context(tc.tile_pool(name="aps", bufs=1, space="PSUM"))
    araw_pool = ctx.enter_context(tc.tile_pool(name="araw", bufs=1, space="PSUM"))
    p_pool = ctx.enter_context(tc.tile_pool(name="pps", bufs=2, space="PSUM"))
    s_psum_pool = ctx.enter_context(tc.tile_pool(name="spsum", bufs=1, space="PSUM"))
    sb2_pool = ctx.enter_context(tc.tile_pool(name="sb2", bufs=6))

    ident = const_pool.tile([128, 128], BF, tag="ident")
    make_identity(nc, ident[:, :])
    ones_row = const_pool.tile([1, 128], F32, tag="ones_row")
    nc.gpsimd.memset(ones_row[:, :], 1.0)

    # causal keep-mask (s <= l), duplicated along cols for both branches
    maskA = const_pool.tile([128, 256], F32, tag="maskA")
    nc.gpsimd.memset(maskA[:, :], 1.0)
    nc.gpsimd.affine_select(
        out=maskA[:, :], in_=maskA[:, :], compare_op=ALU.is_ge, fill=0.0,
        base=0, pattern=[[0, 2], [1, 128]], channel_multiplier=-1)

    # band mask: cols 0..127: 1 where 0 <= l-s <= BW-1 ; cols 128..136:
    # boundary block: keep prev-chunk key p for query l iff p >= (C-TB)+l
    maskB = const_pool.tile([128, 128 + TB], BF, tag="maskB")
    nc.gpsimd.memset(maskB[:, :], 0.0)
    nc.gpsimd.memset(maskB[:, 0:128], 1.0)
    nc.gpsimd.affine_select(
        out=maskB[:, 0:128], in_=maskB[:, 0:128], compare_op=ALU.is_ge,
        fill=0.0, base=0, pattern=[[1, 128]], channel_multiplier=-1)
    nc.gpsimd.affine_select(
        out=maskB[:, 0:128], in_=maskB[:, 0:128], compare_op=ALU.is_ge,
        fill=0.0, base=BW - 1, pattern=[[-1, 128]], channel_multiplier=1)
    nc.gpsimd.memset(maskB[:, 128 : 128 + TB], 1.0)
    nc.gpsimd.affine_select(
        out=maskB[:, 128 : 128 + TB], in_=maskB[:, 128 : 128 + TB],
        compare_op=ALU.is_ge, fill=0.0, base=-(C - TB), pattern=[[-1, TB]],
        channel_multiplier=1)

    consts = (ident, maskA, maskB, ones_row)
    pools = (fpool, spool, qkt_pool, a_pool, araw_pool, p_pool, s_psum_pool,
             sb2_pool)
    units = []
    for u in range(UPC):
        units.append(_Unit(tc, pools, consts,
                           ins["q"][u], ins["k"][u], ins["v"][u], ins["w"][u],
                           outs["o"][u], tag=u))
    for unit in units:
        unit.prelude(tc)
    # interleave the two units' chunk scans so independent work fills the
    # pipeline bubbles of each unit's serial chain
    for c0 in range(0, NCH, 2):
        for unit in units:
            unit.pair(tc, c0)
    for unit in units:
        unit.store(tc)


# ---------------------------------------------------------------------------
# host-side entry point
#
# run_bass_kernel_spmd builds a fresh jax.jit(shard_map(...)) closure on
# every call, so each call re-traces, re-lowers, and re-loads the NEFF
# (~400 ms host overhead for a ~100 us device kernel). Instead we AOT
# compile the same shard_map dispatch once and cache the Compiled object;
# repeat calls are pure dispatch + transfers.
# ---------------------------------------------------------------------------

_CACHE = {}


def _get_nc():
    if "nc" in _CACHE:
        return _CACHE["nc"]
    nc = bacc.Bacc("TRN2", target_bir_lowering=False, debug=False,
                   enable_asserts=True, num_devices=NCORES)
    ins = {
        "q": nc.dram_tensor("q", [UPC, L, E], F32, kind="ExternalInput").ap(),
        "k": nc.dram_tensor("k", [UPC, L, E], F32, kind="ExternalInput").ap(),
        "v": nc.dram_tensor("v", [UPC, L, E], F32, kind="ExternalInput").ap(),
        "w": nc.dram_tensor("w", [UPC, 3, E], F32, kind="ExternalInput").ap(),
    }
    outs = {"o": nc.dram_tensor("o", [UPC, L, E], F32, kind="ExternalOutput").ap()}
    with tile.TileContext(nc) as tc:
        with ExitStack() as ctx:
            build_core_kernel(ctx, tc, outs, ins)
    nc.compile()
    _CACHE["nc"] = nc
    return nc


def _get_rt():
    if "rt" in _CACHE:
        return _CACHE["rt"]
    import jax
    from jax.experimental.shard_map import shard_map
    from jax.sharding import Mesh, NamedSharding, PartitionSpec

    from concourse import bass2jax

    nc = _get_nc()
    bass2jax.install_neuronx_cc_hook()

    partition_name = (nc.partition_id_tensor.name
                      if nc.partition_id_tensor is not None else None)
    in_names, in_specs_pc = [], {}
    out_names, out_avals = [], []
    for alloc in nc.m.functions[0].allocations:
        if not isinstance(alloc, mybir.MemoryLocationSet):
            continue
        assert alloc.memorylocations
        name = alloc.memorylocations[0].name
        if alloc.kind == "ExternalInput":
            if name != partition_name:
                in_names.append(name)
                in_specs_pc[name] = (tuple(alloc.tensor_shape),
                                     mybir.dt.np(alloc.dtype))
        elif alloc.kind == "ExternalOutput":
            assert alloc.tensor_shape is not None and alloc.dtype is not None
            out_names.append(name)
            out_avals.append(jax.core.ShapedArray(
                tuple(alloc.tensor_shape), mybir.dt.np(alloc.dtype)))
    # dbg_addr (if present) is an ExternalInput with no caller-visible data;
    # it is fed zeros, same as run_bass_via_pjrt does.
    dbg_name = nc.dbg_addr.name if nc.dbg_addr is not None else None
    if dbg_name is not None and dbg_name not in in_specs_pc:
        in_names.append(dbg_name)
        in_specs_pc[dbg_name] = ((1, 2), np.uint32)

    n_params = len(in_names)
    n_outs = len(out_names)
    all_in = tuple(in_names + out_names +
                   ([partition_name] if partition_name else []))
    donate = tuple(range(n_params, n_params + n_outs))

    devices = jax.devices()[:NCORES]
    assert len(devices) == NCORES
    mesh = Mesh(np.asarray(devices), ("core",))

    def _body(*args):
        operands = list(args)
        if partition_name is not None:
            operands.append(bass2jax.partition_id_tensor())
        outs = bass2jax._bass_exec_p.bind(
            *operands,
            out_avals=tuple(out_avals),
            in_names=all_in,
            out_names=tuple(out_names),
            lowering_input_output_aliases=(),
            sim_require_finite=True,
            sim_require_nnan=True,
            nc=nc,
        )
        return tuple(outs)

    in_specs = (PartitionSpec("core"),) * (n_params + n_outs)
    out_specs = (PartitionSpec("core"),) * n_outs

    def g(shape, dt):
        return jax.ShapeDtypeStruct((NCORES * shape[0], *shape[1:]), dt)

    lower_args = ([g(*in_specs_pc[n]) for n in in_names] +
                  [g(a.shape, a.dtype) for a in out_avals])

    def compile_fn():
        fn = jax.jit(
            shard_map(_body, mesh=mesh, in_specs=in_specs,
                      out_specs=out_specs, check_rep=False),
            donate_argnums=donate, keep_unused=True)
        return fn.lower(*lower_args).compile()

    try:
        fn = bass2jax.fast_dispatch_compile(compile_fn)
    except Exception:
        fn = jax.jit(
            shard_map(_body, mesh=mesh, in_specs=in_specs,
                      out_specs=out_specs, check_rep=False),
            donate_argnums=donate, keep_unused=True)

    # donated output buffers are produced on-device (no H2B zero upload);
    # dbg zeros (64B) are cached on device once — not donated, so reusable.
    shard0 = NamedSharding(mesh, PartitionSpec("core"))
    zfns = [jax.jit(
        (lambda sh, dt: (lambda: jax.numpy.zeros(sh, dt)))(
            (NCORES * a.shape[0], *a.shape[1:]), a.dtype),
        out_shardings=shard0) for a in out_avals]
    dev_const = {}
    if dbg_name is not None:
        dev_const[dbg_name] = jax.device_put(
            np.zeros((NCORES, 2), np.uint32), shard0)

    rt = dict(fn=fn, in_names=in_names, out_names=out_names,
              zfns=zfns, dev_const=dev_const)
    _CACHE["rt"] = rt
    return rt


def kernel(**inputs):
    rt = _get_rt()
    queries = np.asarray(inputs["queries"], dtype=np.float32)
    keys = np.asarray(inputs["keys"], dtype=np.float32)
    values = np.asarray(inputs["values"], dtype=np.float32)
    W1 = np.asarray(inputs["W1"], dtype=np.float32)
    W2 = np.asarray(inputs["W2"], dtype=np.float32)
    W3 = np.asarray(inputs["W3"], dtype=np.float32)

    # unit u = n*H + h lives at global row u (core u//UPC, slot u%UPC)
    def shard(x):
        return np.ascontiguousarray(
            x.transpose(0, 2, 1, 3).reshape(N * H, L, E))

    wg = np.ascontiguousarray(np.tile(
        np.stack([W1[0, 0], W2[0, 0], W3[0, 0]], axis=1), (N, 1, 1)))
    args = {"q": shard(queries), "k": shard(keys), "v": shard(values),
            "w": wg, **rt["dev_const"]}
    ordered = [args[n] for n in rt["in_names"]]
    zeros = [zf() for zf in rt["zfns"]]
    outs = rt["fn"](*ordered, *zeros)
    _CACHE["last_results"] = None
    o = np.asarray(outs[0])  # [N*H, L, E]
    return np.ascontiguousarray(
        o.reshape(N, H, L, E).transpose(0, 2, 1, 3))

